# revision 46
# baseline (speedup 1.0000x reference)
"""Trainium2 Bass kernel for nn_BasisLinear (B=65536, Cin=64, Cout=64, Rin=Rout=4, R=16).

The module computes, per batch row b:
    out[b, O, p] = sum_{I,q} W[O,p,I,q] * x[b,I,q] + bias[O,p]
with W = einsum('rpq,rOI->OpIq', basis, coeffs) a tiny [256, 256] matrix and
bias = einsum('rp,rO->Op') a [256] vector — i.e. a plain 256->256 linear layer
over the flattened feature dim, batch 65536.

Strategy (data-parallel over batch across 8 cores, per the sharding hint).
The correctness gate is rel_err < 2e-2, so everything runs single-plane
bf16 (x, W, and the stored output), which halves HBM traffic vs fp32 and
cuts matmul work 3x vs the hi/lo-split scheme; measured rel err ~4e-3.

  * Host folds basis/coeffs into W^T [256(f_in), 256(f_out)] bf16 and fp32
    bias; shards x into 8 x [8192, 256], transposes each shard to put f_in
    on partitions, rounds to bf16, and lays it out chunk-blocked so every
    chunk loads with one dma_start of contiguous-per-partition descriptors.
  * Device (per core): x (4 MiB) is fully SBUF-resident.  ALL chunk loads
    are issued up-front on the sync HWDGE ring in compute order — per-ring
    FIFO means data arrives in exactly the order the PE consumes it.  The
    first dma carries weights+bias+chunk0 together (one completion receipt
    gates the first real matmul).  ~5 cold N=512 warm-up matmuls run while
    the first load is in flight so the HAM clock-gate (1.2 -> 2.4 GHz)
    releases right as real work starts.
  * psum[f_out, b] accumulates w_tile^T @ x_tile over the two K-halves
    (K = 256 = 2 x 128 partitions, f_out = 256 = 2 x 128-row tiles mi0/mi1,
    moving N = 512 batch columns/matmul).  Each chunk's two PSUM tiles
    evacuate with ONE op each — mi0 on DVE, mi1 on ACT, concurrently —
    adding the bias and rounding to bf16 on the way to SBUF.
  * Stores (bf16) ride the ACT HWDGE ring early (sync is still issuing
    loads) and the sync ring mid-drain; the last four alternate rings so
    the exec-critical tail stores issue concurrently.
  * Host unpacks the store layout back to [65536, 64, 4] fp32.
"""

import numpy as np
import ml_dtypes

import concourse.bacc as bacc
import concourse.mybir as mybir
import concourse.tile as tile
from concourse import bass_utils

N_CORES = 8
B = 65536
F = 256            # Cin*Rin == Cout*Rout
B_CORE = B // N_CORES

CHUNK = 1024       # batch columns per DMA chunk
SUB = 512          # moving free dim per matmul (fp32 max)
MODE = "bf16x1"


def _planes(mode):
    return 2 if mode == "bf16x3" else 1   # x planes (hi/lo) per k-half


def _chunk_sizes(chunk, b_core):
    """Small head chunks so the PE starts early, uniform 1024s in the
    middle, small tail so the exec-critical final store is short.  All
    loads ride ONE ring in this order, so data ARRIVES in compute order
    (per-ring FIFO) at ~the rate the PE consumes it."""
    del chunk
    head, tail = [256, 256, 512], [512, 256, 256]
    rest = b_core - sum(head) - sum(tail)
    assert rest % 1024 == 0
    return head + [1024] * (rest // 1024) + tail


def _sub_sizes(sc):
    return [SUB] * (sc // SUB) if sc >= SUB else [sc]


def _store_sizes(sc, c, chunks):
    """<=1024-col store slices; the kernel's very last store is 2x512 so
    the final store transfer (on the exec-time critical tail) is short."""
    if sc <= 1024:
        return [sc]
    sizes = [1024] * (sc // 1024)
    if c == len(chunks) - 1:
        sizes[-1:] = [512, 512]
    return sizes


def build_program(mode=MODE, chunk=CHUNK, b_core=B_CORE):
    """Build + compile the SPMD Bass program (same NEFF on all 8 cores)."""
    f32 = mybir.dt.float32
    bf16 = mybir.dt.bfloat16
    f32r = mybir.dt.float32r

    n_xp = _planes(mode)                  # 1 or 2 x planes
    n_pl = 2 * n_xp                       # (plane = xp*2 + ki)
    n_w = 2 if mode == "bf16x3" else 1    # weight planes
    if mode in ("bf16x3", "bf16x1"):
        mm_dt, x_dt = bf16, bf16
    elif mode == "f32":
        mm_dt, x_dt = f32, f32
    else:
        mm_dt, x_dt = f32r, f32r
    o_dt = bf16 if mode == "bf16x1" else f32   # stored-output dtype

    nc = bacc.Bacc("TRN2", target_bir_lowering=False, debug=False,
                   num_devices=N_CORES)

    # x chunk-blocked, xp-major: xpk[p, xp, 2*col0 + ki*sc + j] for chunk at
    # col0 (size sc) => one 2*sc-contiguous run per (xp, chunk) per partition
    xpk = nc.dram_tensor("xpk", (128, n_xp, 2 * b_core), x_dt,
                         kind="ExternalInput")
    # constants: n_w weight planes as [128, 2F] (in units of mm_dt) plus the
    # fp32 bias [128, 2] appended bit-identically in the pack dtype.
    # They ride together with chunk 0 in ONE dram tensor -> ONE dma ->
    # ONE completion receipt gating the first real matmul.
    pack_dt = bf16 if mode in ("bf16x3", "bf16x1") else mm_dt
    bias_cols = 4 if mode in ("bf16x3", "bf16x1") else 2
    wcols = n_w * 2 * F + bias_cols
    c0sz = _chunk_sizes(chunk, b_core)[0]
    xw0 = nc.dram_tensor("xw0", (128, wcols + n_pl * c0sz), x_dt,
                         kind="ExternalInput")
    # out: [128, 2*b] = per chunk one [2(mi), sc] contiguous block
    outT = nc.dram_tensor("outT", (128, 2 * b_core), o_dt,
                          kind="ExternalOutput")

    chunks = _chunk_sizes(chunk, b_core)
    n_ch = len(chunks)

    with tile.TileContext(nc) as tc:
        with (
            tc.tile_pool(name="consts", bufs=1) as consts,
            tc.tile_pool(name="xbuf", bufs=1) as xbuf,
            tc.tile_pool(name="obuf", bufs=1) as obuf,
            tc.tile_pool(name="psum", bufs=4, space="PSUM") as psum,
        ):
            # x is small enough to be fully SBUF-resident: issue EVERY
            # chunk load up-front, ALL on the sync ring in compute order —
            # per-ring FIFO means data lands in exactly the order the PE
            # consumes it, at ~the PE's warm consumption rate.  The first
            # dma carries weights+bias+chunk0 in one shot.
            xw0_sb = consts.tile([128, wcols + n_pl * c0sz], x_dt)
            nc.sync.dma_start(out=xw0_sb[:], in_=xw0.ap())
            wpack_sb = xw0_sb[:, :wcols]
            x_sbs = [xw0_sb[:, wcols:]]
            col0 = chunks[0]
            for c, sc in enumerate(chunks[1:], start=1):
                x_sb = xbuf.tile([128, n_pl * sc], x_dt, tag=f"x{c}",
                                 bufs=1, name=f"x_{c}")
                nc.sync.dma_start(
                    out=x_sb.rearrange("p (xp r) -> p xp r", xp=n_xp),
                    in_=xpk.ap()[:, :, 2 * col0:2 * (col0 + sc)])
                x_sbs.append(x_sb)
                col0 += sc

            w_sbs = [
                wpack_sb[:, wi * 2 * F:(wi + 1) * 2 * F]
                for wi in range(n_w)
            ]
            bias_sb = wpack_sb[:, n_w * 2 * F:
                               n_w * 2 * F + bias_cols].bitcast(f32)

            # PE warm-up with a dependency-free source (memset, not the
            # const DMA): cold N=512 matmuls spanning PAST the first load's
            # completion receipt under jitter, so the HAM busy-streak never
            # breaks between warm-up and real work — the 1.2 -> 2.4 GHz
            # unthrottle then lands deterministically ~3.4us in, and real
            # matmuls start at full clock.
            wu_src = consts.tile([128, SUB], mm_dt)
            nc.vector.memset(wu_src[:], 1.0)
            n_wu = 7
            for i in range(n_wu):
                wu_ps = psum.tile([128, 2 * SUB], f32, tag="ps",
                                  name=f"wu_{i}")
                nc.tensor.matmul(wu_ps[:, :SUB], wu_src[:, :128],
                                 wu_src[:], start=True, stop=True)

            # dummy ACT op with no deps: hoists the one-time ACT_TABLE_LOAD
            # off the first-evacuation critical path (after the load issues)
            dummy = consts.tile([128, 1], f32)
            nc.vector.memset(dummy[:], 0.0)
            nc.scalar.add(out=dummy[:], in_=dummy[:], add=1.0)

            # (x_plane, w_plane) matmul terms accumulated into psum
            if mode == "bf16x3":
                terms = ((0, 0), (0, 1), (1, 0))   # xh*wh + xh*wl + xl*wh
            else:
                terms = ((0, 0),)                  # bf16x1: xh*wh only

            col0 = 0       # batch-column offset of the current chunk
            out_off = 0    # column offset into outT
            for c, sc in enumerate(chunks):
                x_sb = x_sbs[c]
                o_sb = obuf.tile([128, 2 * sc], o_dt, tag=f"o{c}",
                                 bufs=1, name=f"o_{c}")

                def x_ap(xp, ki, ssl):
                    base = (xp * 2 + ki) * sc
                    return x_sb[:, base + ssl.start: base + ssl.stop]

                # one chunk = one block: a PAIR of two-bank [128, sc<=1024]
                # PSUM tiles (mi0/mi1).  Each tile is evacuated by ONE op —
                # mi0 on DVE, mi1 on ACT, concurrently — so evac keeps pace
                # with the warm PE at half the instruction count.
                ps_tiles = [
                    psum.tile([128, 2 * SUB], f32, tag="ps",
                              name=f"ps_{c}_{mi}")
                    for mi in range(2)
                ]
                pss = [t[:, 0:sc] for t in ps_tiles]
                first, last = terms[0], terms[-1]
                soff = 0
                for si, ssz in enumerate(_sub_sizes(sc)):
                    ssl = slice(soff, soff + ssz)
                    for ki in range(2):
                        for t in terms:
                            xp, wp = t
                            for mi in range(2):
                                w_ap = w_sbs[wp][:, ki * F + mi * 128:
                                                 ki * F + (mi + 1) * 128]
                                nc.tensor.matmul(
                                    pss[mi][:, soff:soff + ssz], w_ap,
                                    x_ap(xp, ki, ssl),
                                    start=(ki == 0 and t == first),
                                    stop=(ki == 1 and t == last))
                    soff += ssz
                # o_sb layout per chunk: [2(mi), sc] per partition.  The
                # drain is evac-paced, so balance the two engines' per-block
                # serial time: DVE runs at 0.96 GHz vs ACT's 1.2, so for
                # full 1024-blocks ACT takes a 32-col sliver of mi0 on top
                # of all of mi1 — both engines then finish in ~1.16us
                # instead of pacing at DVE's 1.28us.
                xsplit = sc - 32 if sc == 2 * SUB else sc
                nc.vector.tensor_scalar_add(
                    out=o_sb[:, 0:xsplit], in0=pss[0][:, 0:xsplit],
                    scalar1=bias_sb[:, 0:1])
                if xsplit < sc:
                    nc.scalar.add(out=o_sb[:, xsplit:sc],
                                  in_=pss[0][:, xsplit:sc],
                                  add=bias_sb[:, 0:1])
                nc.scalar.add(out=o_sb[:, sc:2 * sc],
                              in_=pss[1],
                              add=bias_sb[:, 1:2])
                # stores: early chunks on scalar (sync is still issuing
                # loads), mid chunks on sync (keeping stores OFF scalar
                # protects the ACT evac cadence mid-drain), and the last
                # FOUR alternate rings so the exec-critical tail stores
                # issue concurrently instead of serializing behind sync's
                # backpressured descriptor generation
                if c < 3:
                    st_eng = nc.scalar
                elif c >= len(chunks) - 4:
                    st_eng = nc.scalar if (len(chunks) - 1 - c) % 2 == 0 \
                        else nc.sync
                else:
                    st_eng = nc.sync
                if c == len(chunks) - 1:
                    # last chunk: one store per mi half on DIFFERENT rings,
                    # each firing the moment its own evac lands
                    nc.scalar.dma_start(
                        out=outT.ap()[:, out_off: out_off + sc],
                        in_=o_sb[:, 0:sc])
                    nc.sync.dma_start(
                        out=outT.ap()[:, out_off + sc: out_off + 2 * sc],
                        in_=o_sb[:, sc:2 * sc])
                    out_off += 2 * sc
                else:
                    st_off = 0
                    for stz in _store_sizes(sc, c, chunks):
                        st_eng.dma_start(
                            out=outT.ap()[:, out_off: out_off + 2 * stz],
                            in_=o_sb[:, 2 * st_off: 2 * (st_off + stz)])
                        out_off += 2 * stz
                        st_off += stz
                col0 += sc

    nc.compile()
    return nc


def round_fp32r(a):
    """Round-to-nearest-even to 11 mantissa bits (matches hw fp32r)."""
    u = a.view(np.uint32)
    keep = np.uint32(0xFFFFF000)
    lsb = (u >> np.uint32(12)) & np.uint32(1)
    r = (u + np.uint32(0x7FF) + lsb) & keep
    return r.view(np.float32)


def split_bf16(a):
    """a (fp32) -> (hi, lo) bf16 with hi + lo ≈ a to ~16 mantissa bits."""
    hi = a.astype(ml_dtypes.bfloat16)
    lo = (a - hi.astype(np.float32)).astype(ml_dtypes.bfloat16)
    return hi, lo


def host_prepack(basis, coeffs, basis_bias, coeffs_bias):
    """Fold the basis factorization into wT [256,256] and bias [128,2]."""
    b_sq = np.asarray(basis, np.float32)[:, 0, :, 0, :]     # [R, p, q]
    c_sq = np.asarray(coeffs, np.float32)[:, :, 0, :, 0]    # [R, O, I]
    # W[O,p,I,q] -> flat [f_out, f_in]
    W = np.einsum("rpq,rOI->OpIq", b_sq, c_sq)
    w_flat = np.ascontiguousarray(W.reshape(F, F))
    wT = np.ascontiguousarray(w_flat.T)                     # [f_in, f_out]
    bb = np.asarray(basis_bias, np.float32)[:, 0, :]        # [Rb, p]
    cb = np.asarray(coeffs_bias, np.float32)[:, :, 0]       # [Rb, O]
    bias_vec = np.einsum("rp,rO->Op", bb, cb).reshape(F)    # [f_out]
    bias_mat = np.ascontiguousarray(bias_vec.reshape(2, 128).T)  # [128, 2]
    return wT, bias_mat


def _fold_khalf(w):
    """[256, F] -> [128, 2*F] with w[ki*128+p, f] at [p, ki*F+f]."""
    return np.ascontiguousarray(
        w.reshape(2, 128, F).transpose(1, 0, 2).reshape(128, 2 * F))


def make_in_maps(x, basis, coeffs, basis_bias, coeffs_bias, mode=MODE,
                 chunk=CHUNK, b_core=B_CORE):
    wT, bias_mat = host_prepack(basis, coeffs, basis_bias, coeffs_bias)
    x2 = np.ascontiguousarray(np.asarray(x, np.float32)).reshape(-1, F)
    if mode == "f32r":
        wT = round_fp32r(wT)
        x2 = round_fp32r(x2)
    n_xp = _planes(mode)

    bf = ml_dtypes.bfloat16
    if mode == "bf16x3":
        wh, wl = split_bf16(wT)
        parts = [_fold_khalf(wh).view(np.uint16),
                 _fold_khalf(wl).view(np.uint16),
                 np.ascontiguousarray(bias_mat).view(np.uint16)]
        wpack = np.ascontiguousarray(np.concatenate(parts, axis=1)).view(bf)
    elif mode == "bf16x1":
        wh = wT.astype(bf)
        parts = [_fold_khalf(wh).view(np.uint16),
                 np.ascontiguousarray(bias_mat).view(np.uint16)]
        wpack = np.ascontiguousarray(np.concatenate(parts, axis=1)).view(bf)
    else:
        wpack = np.ascontiguousarray(
            np.concatenate([_fold_khalf(wT), bias_mat], axis=1))

    in_maps = []
    n_cores = x2.shape[0] // b_core
    for c in range(n_cores):
        shard_t = np.ascontiguousarray(
            x2[c * b_core:(c + 1) * b_core].T)              # [F, b_core]
        if mode == "bf16x3":
            planes = split_bf16(shard_t)                    # (xh, xl) [F, b]
            dt = bf
        elif mode == "bf16x1":
            planes = (shard_t.astype(bf),)                  # hi plane only
            dt = bf
        else:
            planes = (shard_t,)
            dt = np.float32
        # xpk[p, xp, 2*col0 + ki*sc + j] = planes[xp][ki*128+p, col0+j]
        xpk = np.empty((128, n_xp, 2 * b_core), dt)
        for xp, pl in enumerate(planes):
            col0 = 0
            for sc in _chunk_sizes(chunk, b_core):
                blk = pl[:, col0:col0 + sc].reshape(2, 128, sc)
                xpk[:, xp, 2 * col0:2 * col0 + sc] = blk[0]
                xpk[:, xp, 2 * col0 + sc:2 * (col0 + sc)] = blk[1]
                col0 += sc
        # xw0 = [wpack | chunk0 planes xp-major]: the one first-dma tensor
        c0sz = _chunk_sizes(chunk, b_core)[0]
        chunk0 = xpk[:, :, :2 * c0sz].reshape(128, n_xp * 2 * c0sz)
        xw0 = np.ascontiguousarray(
            np.concatenate([wpack.view(dt), chunk0], axis=1))
        in_maps.append({"xpk": xpk, "xw0": xw0})
    return in_maps


def assemble_out(results, chunk=CHUNK, b_core=B_CORE):
    sizes = _chunk_sizes(chunk, b_core)      # one [2(mi), sc] block/chunk
    n_cores = len(results)
    out = np.empty((n_cores * b_core, F), np.float32)
    for c in range(n_cores):
        o = results[c]["outT"]                  # [128, 2*b_core]
        row, off = c * b_core, 0
        for s in sizes:
            blk = o[:, off:off + 2 * s].reshape(128, 2, s)
            # out[row+j, mi*128+p] = blk[p, mi, j]
            out[row:row + s] = blk.transpose(2, 1, 0).reshape(s, F)
            row += s
            off += 2 * s
    return out


_PROGRAM = None


def kernel(x, basis, coeffs, basis_bias, coeffs_bias):
    global _PROGRAM
    if _PROGRAM is None:
        _PROGRAM = build_program()
    in_maps = make_in_maps(x, basis, coeffs, basis_bias, coeffs_bias)
    res = bass_utils.run_bass_kernel_spmd(
        _PROGRAM, in_maps, core_ids=list(range(N_CORES)))
    return assemble_out(res.results).reshape(B, 64, 4)



# revision 48
# speedup vs baseline: 1.0173x; 1.0173x over previous
"""Trainium2 Bass kernel for nn_BasisLinear (B=65536, Cin=64, Cout=64, Rin=Rout=4, R=16).

The module computes, per batch row b:
    out[b, O, p] = sum_{I,q} W[O,p,I,q] * x[b,I,q] + bias[O,p]
with W = einsum('rpq,rOI->OpIq', basis, coeffs) a tiny [256, 256] matrix and
bias = einsum('rp,rO->Op') a [256] vector — i.e. a plain 256->256 linear layer
over the flattened feature dim, batch 65536.

Strategy (data-parallel over batch across 8 cores, per the sharding hint).
The correctness gate is rel_err < 2e-2, so everything runs single-plane
bf16 (x, W, and the stored output), which halves HBM traffic vs fp32 and
cuts matmul work 3x vs the hi/lo-split scheme; measured rel err ~4e-3.

  * Host folds basis/coeffs into W^T [256(f_in), 256(f_out)] bf16 and fp32
    bias; shards x into 8 x [8192, 256], transposes each shard to put f_in
    on partitions, rounds to bf16, and lays it out chunk-blocked so every
    chunk loads with one dma_start of contiguous-per-partition descriptors.
  * Device (per core): x (4 MiB) is fully SBUF-resident.  ALL chunk loads
    are issued up-front on the sync HWDGE ring in compute order — per-ring
    FIFO means data arrives in exactly the order the PE consumes it.  The
    first dma carries weights+bias+chunk0 together (one completion receipt
    gates the first real matmul).  ~5 cold N=512 warm-up matmuls run while
    the first load is in flight so the HAM clock-gate (1.2 -> 2.4 GHz)
    releases right as real work starts.
  * psum[f_out, b] accumulates w_tile^T @ x_tile over the two K-halves
    (K = 256 = 2 x 128 partitions, f_out = 256 = 2 x 128-row tiles mi0/mi1,
    moving N = 512 batch columns/matmul).  Each chunk's two PSUM tiles
    evacuate with ONE op each — mi0 on DVE, mi1 on ACT, concurrently —
    adding the bias and rounding to bf16 on the way to SBUF.
  * Stores (bf16) ride the ACT HWDGE ring early (sync is still issuing
    loads) and the sync ring mid-drain; the last four alternate rings so
    the exec-critical tail stores issue concurrently.
  * Host unpacks the store layout back to [65536, 64, 4] fp32.
"""

import numpy as np
import ml_dtypes

import concourse.bacc as bacc
import concourse.mybir as mybir
import concourse.tile as tile
from concourse import bass_utils

N_CORES = 8
B = 65536
F = 256            # Cin*Rin == Cout*Rout
B_CORE = B // N_CORES

CHUNK = 1024       # batch columns per DMA chunk
SUB = 512          # moving free dim per matmul (fp32 max)
MODE = "bf16x1"


def _planes(mode):
    return 2 if mode == "bf16x3" else 1   # x planes (hi/lo) per k-half


def _chunk_sizes(chunk, b_core):
    """Small head chunks so the PE starts early, uniform 1024s in the
    middle, small tail so the exec-critical final store is short.  All
    loads ride ONE ring in this order, so data ARRIVES in compute order
    (per-ring FIFO) at ~the rate the PE consumes it."""
    del chunk
    head, tail = [256, 256, 512], [512, 256, 256]
    rest = b_core - sum(head) - sum(tail)
    assert rest % 1024 == 0
    return head + [1024] * (rest // 1024) + tail


def _sub_sizes(sc):
    return [SUB] * (sc // SUB) if sc >= SUB else [sc]


def _store_sizes(sc, c, chunks):
    """<=1024-col store slices; the kernel's very last store is 2x512 so
    the final store transfer (on the exec-time critical tail) is short."""
    if sc <= 1024:
        return [sc]
    sizes = [1024] * (sc // 1024)
    if c == len(chunks) - 1:
        sizes[-1:] = [512, 512]
    return sizes


def build_program(mode=MODE, chunk=CHUNK, b_core=B_CORE):
    """Build + compile the SPMD Bass program (same NEFF on all 8 cores)."""
    f32 = mybir.dt.float32
    bf16 = mybir.dt.bfloat16
    f32r = mybir.dt.float32r

    n_xp = _planes(mode)                  # 1 or 2 x planes
    n_pl = 2 * n_xp                       # (plane = xp*2 + ki)
    n_w = 2 if mode == "bf16x3" else 1    # weight planes
    if mode in ("bf16x3", "bf16x1"):
        mm_dt, x_dt = bf16, bf16
    elif mode == "f32":
        mm_dt, x_dt = f32, f32
    else:
        mm_dt, x_dt = f32r, f32r
    o_dt = bf16 if mode == "bf16x1" else f32   # stored-output dtype

    nc = bacc.Bacc("TRN2", target_bir_lowering=False, debug=False,
                   num_devices=N_CORES)

    # x chunk-blocked, xp-major: xpk[p, xp, 2*col0 + ki*sc + j] for chunk at
    # col0 (size sc) => one 2*sc-contiguous run per (xp, chunk) per partition
    xpk = nc.dram_tensor("xpk", (128, n_xp, 2 * b_core), x_dt,
                         kind="ExternalInput")
    # constants: n_w weight planes as [128, 2F] (in units of mm_dt) plus the
    # fp32 bias [128, 2] appended bit-identically in the pack dtype.
    # They ride together with chunk 0 in ONE dram tensor -> ONE dma ->
    # ONE completion receipt gating the first real matmul.
    pack_dt = bf16 if mode in ("bf16x3", "bf16x1") else mm_dt
    bias_cols = 4 if mode in ("bf16x3", "bf16x1") else 2
    wcols = n_w * 2 * F + bias_cols
    c0sz = _chunk_sizes(chunk, b_core)[0]
    xw0 = nc.dram_tensor("xw0", (128, wcols + n_pl * c0sz), x_dt,
                         kind="ExternalInput")
    # out: [128, 2*b] = per chunk one [2(mi), sc] contiguous block
    outT = nc.dram_tensor("outT", (128, 2 * b_core), o_dt,
                          kind="ExternalOutput")

    chunks = _chunk_sizes(chunk, b_core)
    n_ch = len(chunks)

    with tile.TileContext(nc) as tc:
        with (
            tc.tile_pool(name="consts", bufs=1) as consts,
            tc.tile_pool(name="xbuf", bufs=1) as xbuf,
            tc.tile_pool(name="obuf", bufs=1) as obuf,
            tc.tile_pool(name="psum", bufs=4, space="PSUM") as psum,
        ):
            # x is small enough to be fully SBUF-resident: issue EVERY
            # chunk load up-front, ALL on the sync ring in compute order —
            # per-ring FIFO means data lands in exactly the order the PE
            # consumes it, at ~the PE's warm consumption rate.  The first
            # dma carries weights+bias+chunk0 in one shot.
            xw0_sb = consts.tile([128, wcols + n_pl * c0sz], x_dt)
            nc.sync.dma_start(out=xw0_sb[:], in_=xw0.ap())
            wpack_sb = xw0_sb[:, :wcols]
            x_sbs = [xw0_sb[:, wcols:]]
            col0 = chunks[0]
            for c, sc in enumerate(chunks[1:], start=1):
                x_sb = xbuf.tile([128, n_pl * sc], x_dt, tag=f"x{c}",
                                 bufs=1, name=f"x_{c}")
                nc.sync.dma_start(
                    out=x_sb.rearrange("p (xp r) -> p xp r", xp=n_xp),
                    in_=xpk.ap()[:, :, 2 * col0:2 * (col0 + sc)])
                x_sbs.append(x_sb)
                col0 += sc

            w_sbs = [
                wpack_sb[:, wi * 2 * F:(wi + 1) * 2 * F]
                for wi in range(n_w)
            ]
            bias_sb = wpack_sb[:, n_w * 2 * F:
                               n_w * 2 * F + bias_cols].bitcast(f32)

            # PE warm-up with a dependency-free source (memset, not the
            # const DMA): cold N=512 matmuls spanning PAST the first load's
            # completion receipt under jitter, so the HAM busy-streak never
            # breaks between warm-up and real work — the 1.2 -> 2.4 GHz
            # unthrottle then lands deterministically ~3.4us in, and real
            # matmuls start at full clock.
            wu_src = consts.tile([128, SUB], mm_dt)
            nc.vector.memset(wu_src[:], 1.0)
            n_wu = 7
            for i in range(n_wu):
                wu_ps = psum.tile([128, 2 * SUB], f32, tag="ps",
                                  name=f"wu_{i}")
                nc.tensor.matmul(wu_ps[:, :SUB], wu_src[:, :128],
                                 wu_src[:], start=True, stop=True)

            # dummy ACT op with no deps: hoists the one-time ACT_TABLE_LOAD
            # off the first-evacuation critical path (after the load issues)
            dummy = consts.tile([128, 1], f32)
            nc.vector.memset(dummy[:], 0.0)
            nc.scalar.add(out=dummy[:], in_=dummy[:], add=1.0)

            # (x_plane, w_plane) matmul terms accumulated into psum
            if mode == "bf16x3":
                terms = ((0, 0), (0, 1), (1, 0))   # xh*wh + xh*wl + xl*wh
            else:
                terms = ((0, 0),)                  # bf16x1: xh*wh only

            col0 = 0       # batch-column offset of the current chunk
            out_off = 0    # column offset into outT
            for c, sc in enumerate(chunks):
                x_sb = x_sbs[c]
                o_sb = obuf.tile([128, 2 * sc], o_dt, tag=f"o{c}",
                                 bufs=1, name=f"o_{c}")

                def x_ap(xp, ki, ssl):
                    base = (xp * 2 + ki) * sc
                    return x_sb[:, base + ssl.start: base + ssl.stop]

                # one chunk = one block: a PAIR of two-bank [128, sc<=1024]
                # PSUM tiles (mi0/mi1).  Each tile is evacuated by ONE op —
                # mi0 on DVE, mi1 on ACT, concurrently — so evac keeps pace
                # with the warm PE at half the instruction count.
                ps_tiles = [
                    psum.tile([128, 2 * SUB], f32, tag="ps",
                              name=f"ps_{c}_{mi}")
                    for mi in range(2)
                ]
                pss = [t[:, 0:sc] for t in ps_tiles]
                first, last = terms[0], terms[-1]
                soff = 0
                for si, ssz in enumerate(_sub_sizes(sc)):
                    ssl = slice(soff, soff + ssz)
                    for ki in range(2):
                        for t in terms:
                            xp, wp = t
                            for mi in range(2):
                                w_ap = w_sbs[wp][:, ki * F + mi * 128:
                                                 ki * F + (mi + 1) * 128]
                                nc.tensor.matmul(
                                    pss[mi][:, soff:soff + ssz], w_ap,
                                    x_ap(xp, ki, ssl),
                                    start=(ki == 0 and t == first),
                                    stop=(ki == 1 and t == last))
                    soff += ssz
                # o_sb layout per chunk: [2(mi), sc] per partition
                nc.vector.tensor_scalar_add(
                    out=o_sb[:, 0:sc], in0=pss[0],
                    scalar1=bias_sb[:, 0:1])
                nc.scalar.add(out=o_sb[:, sc:2 * sc],
                              in_=pss[1],
                              add=bias_sb[:, 1:2])
                # stores: early chunks on scalar (sync is still issuing
                # loads), mid chunks on sync (keeping stores OFF scalar
                # protects the ACT evac cadence mid-drain), and the last
                # FOUR alternate rings so the exec-critical tail stores
                # issue concurrently instead of serializing behind sync's
                # backpressured descriptor generation
                if c < 3:
                    st_eng = nc.scalar
                elif c >= len(chunks) - 4:
                    st_eng = nc.scalar if (len(chunks) - 1 - c) % 2 == 0 \
                        else nc.sync
                else:
                    st_eng = nc.sync
                st_off = 0
                for stz in _store_sizes(sc, c, chunks):
                    st_eng.dma_start(
                        out=outT.ap()[:, out_off: out_off + 2 * stz],
                        in_=o_sb[:, 2 * st_off: 2 * (st_off + stz)])
                    out_off += 2 * stz
                    st_off += stz
                col0 += sc

    nc.compile()
    return nc


def round_fp32r(a):
    """Round-to-nearest-even to 11 mantissa bits (matches hw fp32r)."""
    u = a.view(np.uint32)
    keep = np.uint32(0xFFFFF000)
    lsb = (u >> np.uint32(12)) & np.uint32(1)
    r = (u + np.uint32(0x7FF) + lsb) & keep
    return r.view(np.float32)


def split_bf16(a):
    """a (fp32) -> (hi, lo) bf16 with hi + lo ≈ a to ~16 mantissa bits."""
    hi = a.astype(ml_dtypes.bfloat16)
    lo = (a - hi.astype(np.float32)).astype(ml_dtypes.bfloat16)
    return hi, lo


def host_prepack(basis, coeffs, basis_bias, coeffs_bias):
    """Fold the basis factorization into wT [256,256] and bias [128,2]."""
    b_sq = np.asarray(basis, np.float32)[:, 0, :, 0, :]     # [R, p, q]
    c_sq = np.asarray(coeffs, np.float32)[:, :, 0, :, 0]    # [R, O, I]
    # W[O,p,I,q] -> flat [f_out, f_in]
    W = np.einsum("rpq,rOI->OpIq", b_sq, c_sq)
    w_flat = np.ascontiguousarray(W.reshape(F, F))
    wT = np.ascontiguousarray(w_flat.T)                     # [f_in, f_out]
    bb = np.asarray(basis_bias, np.float32)[:, 0, :]        # [Rb, p]
    cb = np.asarray(coeffs_bias, np.float32)[:, :, 0]       # [Rb, O]
    bias_vec = np.einsum("rp,rO->Op", bb, cb).reshape(F)    # [f_out]
    bias_mat = np.ascontiguousarray(bias_vec.reshape(2, 128).T)  # [128, 2]
    return wT, bias_mat


def _fold_khalf(w):
    """[256, F] -> [128, 2*F] with w[ki*128+p, f] at [p, ki*F+f]."""
    return np.ascontiguousarray(
        w.reshape(2, 128, F).transpose(1, 0, 2).reshape(128, 2 * F))


def make_in_maps(x, basis, coeffs, basis_bias, coeffs_bias, mode=MODE,
                 chunk=CHUNK, b_core=B_CORE):
    wT, bias_mat = host_prepack(basis, coeffs, basis_bias, coeffs_bias)
    x2 = np.ascontiguousarray(np.asarray(x, np.float32)).reshape(-1, F)
    if mode == "f32r":
        wT = round_fp32r(wT)
        x2 = round_fp32r(x2)
    n_xp = _planes(mode)

    bf = ml_dtypes.bfloat16
    if mode == "bf16x3":
        wh, wl = split_bf16(wT)
        parts = [_fold_khalf(wh).view(np.uint16),
                 _fold_khalf(wl).view(np.uint16),
                 np.ascontiguousarray(bias_mat).view(np.uint16)]
        wpack = np.ascontiguousarray(np.concatenate(parts, axis=1)).view(bf)
    elif mode == "bf16x1":
        wh = wT.astype(bf)
        parts = [_fold_khalf(wh).view(np.uint16),
                 np.ascontiguousarray(bias_mat).view(np.uint16)]
        wpack = np.ascontiguousarray(np.concatenate(parts, axis=1)).view(bf)
    else:
        wpack = np.ascontiguousarray(
            np.concatenate([_fold_khalf(wT), bias_mat], axis=1))

    in_maps = []
    n_cores = x2.shape[0] // b_core
    for c in range(n_cores):
        shard_t = np.ascontiguousarray(
            x2[c * b_core:(c + 1) * b_core].T)              # [F, b_core]
        if mode == "bf16x3":
            planes = split_bf16(shard_t)                    # (xh, xl) [F, b]
            dt = bf
        elif mode == "bf16x1":
            planes = (shard_t.astype(bf),)                  # hi plane only
            dt = bf
        else:
            planes = (shard_t,)
            dt = np.float32
        # xpk[p, xp, 2*col0 + ki*sc + j] = planes[xp][ki*128+p, col0+j]
        xpk = np.empty((128, n_xp, 2 * b_core), dt)
        for xp, pl in enumerate(planes):
            col0 = 0
            for sc in _chunk_sizes(chunk, b_core):
                blk = pl[:, col0:col0 + sc].reshape(2, 128, sc)
                xpk[:, xp, 2 * col0:2 * col0 + sc] = blk[0]
                xpk[:, xp, 2 * col0 + sc:2 * (col0 + sc)] = blk[1]
                col0 += sc
        # xw0 = [wpack | chunk0 planes xp-major]: the one first-dma tensor
        c0sz = _chunk_sizes(chunk, b_core)[0]
        chunk0 = xpk[:, :, :2 * c0sz].reshape(128, n_xp * 2 * c0sz)
        xw0 = np.ascontiguousarray(
            np.concatenate([wpack.view(dt), chunk0], axis=1))
        in_maps.append({"xpk": xpk, "xw0": xw0})
    return in_maps


def assemble_out(results, chunk=CHUNK, b_core=B_CORE):
    sizes = _chunk_sizes(chunk, b_core)      # one [2(mi), sc] block/chunk
    n_cores = len(results)
    out = np.empty((n_cores * b_core, F), np.float32)
    for c in range(n_cores):
        o = results[c]["outT"]                  # [128, 2*b_core]
        row, off = c * b_core, 0
        for s in sizes:
            blk = o[:, off:off + 2 * s].reshape(128, 2, s)
            # out[row+j, mi*128+p] = blk[p, mi, j]
            out[row:row + s] = blk.transpose(2, 1, 0).reshape(s, F)
            row += s
            off += 2 * s
    return out


_PROGRAM = None


def kernel(x, basis, coeffs, basis_bias, coeffs_bias):
    global _PROGRAM
    if _PROGRAM is None:
        _PROGRAM = build_program()
    in_maps = make_in_maps(x, basis, coeffs, basis_bias, coeffs_bias)
    res = bass_utils.run_bass_kernel_spmd(
        _PROGRAM, in_maps, core_ids=list(range(N_CORES)))
    return assemble_out(res.results).reshape(B, 64, 4)

